# revision 1
# baseline (speedup 1.0000x reference)
"""Trainium2 Bass kernel for the CGCNN model (8-core SPMD, graph-parallel).

v3 strategy:
- Shard graphs (64/core) -> contiguous node ranges via the sorted batch
  vector; relabel local nodes by descending in-degree. Edges live in a ragged
  (block, slot, partition) layout: partition p of block i owns dst node
  i*128+p; padding slots point at a poison table row (f-half -30000 so
  sigmoid(f)=0 kills padded messages).
- Per layer: node projections -> src table slice (bf16) -> AllGather ->
  [8*NPAD, 256] DRAM table.
- Edge phase per block: TWO batched InstDMAGatherAnt calls (int16 indices
  cover <=32768 rows, so slots split into a lo range over rows [0, 32768) and
  a hi range over [R-32768, R); flexible edges in the overlap balance both),
  one DVE pass adds the Pdst broadcast, one fp8 ef matmul per slot pair
  (pair-stacked edge_attr x block-diagonal Wef, K=128), DVE adds PSUM into G
  (f16), per-block batched Tanh(f/2)/Exp, deferred batched Ln(1+e) paired
  across 2 blocks (halves ACT table reloads), msg=(1+th)*sp in place, per-slot
  identity-matmul aggregation, 0.5/deg scaling, PE transpose, +h residual.
- Input-shipping cuts (the harness re-ships parameters per execute at
  ~0.6ms/MB): edge_attr + Wef in fp8-e4m3, gather indices shipped [16, 8S]
  and replicated to [128, 8S] on-device once, onehot/x/W_emb in bf16.
- BatchNorm: masked sums, AllReduce [128,2], fused scale/bias+ReLU, residual.
- Pooling: per-block bf16 matmuls with 1/graph-size one-hot; tiny MLP.
Output: [1,64] per core, concatenated on host.
"""
import os
import sys
import numpy as np

sys.path.insert(0, '/opt/trn_rl_repo')
os.environ.setdefault("NEURON_SCRATCHPAD_PAGE_SIZE", "256")

import ml_dtypes

BF16NP = ml_dtypes.bfloat16
F8NP = ml_dtypes.float8_e4m3

N = 50000
E = 1600000
HID = 128
NGRAPH = 512
NCONV = 4
EDGE_DIM = 64
NCORES = 8
GPC = NGRAPH // NCORES
BN_EPS = 1e-5
POISON_VAL = -30000.0
LO = 32768  # int16 index range per gather
GCHUNK = 2048  # max descriptors per dma_gather call (SWDGE ring safety)

_CACHE = {}


def _host_prep(inputs):
    x = np.asarray(inputs['x'], np.float32)
    ei = np.asarray(inputs['edge_index']).astype(np.int64)
    ea = np.asarray(inputs['edge_attr'], np.float32)
    batch = np.asarray(inputs['batch']).astype(np.int64)
    src, dst = ei[0], ei[1]

    deg = np.bincount(dst, minlength=N)
    node_start = np.searchsorted(batch, np.arange(0, NGRAPH + 1, GPC))
    n_c = np.diff(node_start)
    NPAD = int(np.ceil((n_c.max() + 2) / 128.0) * 128)
    NB = NPAD // 128
    R = NCORES * NPAD
    OFF = R - LO          # hi gather covers rows [OFF, R)
    POISON_LO = NPAD - 1                       # core0 poison row (< OFF)
    POISON_HI = R - 1                          # core7 poison row (>= OFF)
    assert POISON_HI - OFF < LO and NPAD < OFF

    percore = []
    rows_of = np.empty(N, np.int64)
    for c in range(NCORES):
        ids = np.arange(node_start[c], node_start[c + 1])
        order = ids[np.argsort(-deg[ids], kind='stable')]
        percore.append(order)
        rows_of[order] = c * NPAD + np.arange(len(order))

    # --- per-edge bookkeeping: dst row -> (core, block, partition) ---------
    ro = rows_of[dst]
    order_e = np.argsort(ro, kind='stable')
    ro_s = ro[order_e]
    srow_s = rows_of[src[order_e]]            # src table row per edge
    e_s = order_e
    c_e = ro_s // NPAD
    r_loc = ro_s % NPAD
    blk = r_loc // 128
    p_e = r_loc % 128
    # per (core, block, partition) in-degree -> common per-block slot count
    key = (c_e * NB + blk) * 128 + p_e        # flat row id
    nrows = NCORES * NB * 128
    degr = np.bincount(key, minlength=nrows)
    Bi = np.maximum(degr.reshape(NCORES, NB, 128).max(axis=(0, 2)), 2)
    Bi = Bi + (Bi % 2)                         # slot pairs need even totals
    Bi_lo = Bi // 2                            # kept for cache key compat
    Bi_hi = Bi - Bi_lo
    S = int(Bi.sum())
    slot0 = np.concatenate([[0], np.cumsum(Bi)]).astype(np.int64)

    # --- slot assignment: rank within (core, block, partition) row ---------
    o2 = np.argsort(key, kind='stable')
    k2s = key[o2]
    ch = np.concatenate([[True], k2s[1:] != k2s[:-1]])
    first = np.where(ch)[0]
    starts = np.repeat(first, np.diff(np.concatenate([first, [len(k2s)]])))
    rank2 = np.empty(len(key), np.int64)
    rank2[o2] = np.arange(len(k2s)) - starts
    slot = rank2
    assert (slot < Bi[blk]).all()

    # --- int32 gather index tiles: gidx32[c][p, s0+s] = src table row ------
    gidx32 = np.full((NCORES, 128, S), POISON_LO, np.int32)
    gidx32[c_e, p_e, slot0[blk] + slot] = srow_s.astype(np.int32)

    # --- pair-stacked fp8 edge_attr ---------------------------------------
    PH = S // 2
    pslot0 = (slot0 // 2).astype(np.int64)
    eaT2 = np.zeros((NCORES, 128, PH * 128), F8NP)
    ea_f8 = ea.astype(F8NP)
    pair = slot // 2
    podd = slot % 2
    cols2 = (pslot0[blk] + pair) * 128 + p_e
    eaTsrc = ea_f8[e_s].T                      # [64, E]
    for c in range(NCORES):
        m = c_e == c
        for o in (0, 1):
            mo = m & (podd == o)
            eaT2[c][o * 64:(o + 1) * 64, cols2[mo]] = eaTsrc[:, mo]

    # --- layer-0 src term: pair-stacked x[src] + pad indicator (fp8) ------
    # rows 0-8: x feats (even slot), row 9: pad flag (even); rows 10-18/19: odd
    xp2 = np.zeros((NCORES, 20, PH * 128), F8NP)
    xp2[:, 9, :] = 1.0
    xp2[:, 19, :] = 1.0
    x_f8 = x.astype(F8NP)
    xsrcT = x_f8[src[e_s]].T                   # [9, E] in slot order
    for c in range(NCORES):
        m = c_e == c
        for o in (0, 1):
            mo = m & (podd == o)
            xp2[c][o * 10:o * 10 + 9, cols2[mo]] = xsrcT[:, mo]
            xp2[c][o * 10 + 9, cols2[mo]] = 0.0

    # per-core host tensors
    invc = np.zeros((NCORES, 128, NB), np.float32)
    for c in range(NCORES):
        d = np.zeros(NPAD, np.float32)
        d[:n_c[c]] = np.maximum(deg[percore[c]], 1)
        d[n_c[c]:] = 1.0
        invc[c] = (0.5 / d).reshape(NB, 128).T   # 0.5: msg2 = (1+th)*sp

    onehot = np.zeros((NCORES, 128, NB * GPC), BF16NP)
    mask = np.zeros((NCORES, 1, NPAD), BF16NP)
    xT = np.zeros((NCORES, 9, NPAD), BF16NP)
    for c in range(NCORES):
        g_loc = batch[percore[c]] - c * GPC
        gsz = np.bincount(g_loc, minlength=GPC).astype(np.float32)
        oh = np.zeros((NPAD, GPC), np.float32)
        oh[np.arange(n_c[c]), g_loc] = 1.0 / np.maximum(gsz[g_loc], 1.0)
        onehot[c] = oh.reshape(NB, 128, GPC).transpose(1, 0, 2) \
                      .reshape(128, NB * GPC).astype(BF16NP)
        mask[c, 0, :n_c[c]] = 1.0
        xT[c, :, :n_c[c]] = x[percore[c]].T.astype(BF16NP)

    meta = dict(NPAD=NPAD, NB=NB, Bi=Bi.tolist(), Bi_lo=Bi_lo.tolist(),
                Bi_hi=Bi_hi.tolist(), S=S, slot0=slot0.tolist(),
                pslot0=pslot0.tolist())

    # replicated weights
    Wf = np.asarray(inputs['Wf'], np.float32)
    Ws = np.asarray(inputs['Ws'], np.float32)
    bf_ = np.asarray(inputs['bf'], np.float32)
    bs_ = np.asarray(inputs['bs'], np.float32)
    Wsrc = np.concatenate([np.concatenate([Wf[l, HID:2 * HID], Ws[l, HID:2 * HID]], 1)
                           for l in range(NCONV)], 1)          # [128, 4*256]
    Wdst = np.concatenate([np.concatenate([Wf[l, :HID], Ws[l, :HID]], 1)
                           for l in range(NCONV)], 1)          # [128, 4*256]
    Wef2 = np.zeros((128, NCONV * 512), np.float32)
    for l in range(NCONV):
        w = np.concatenate([Wf[l, 2 * HID:], Ws[l, 2 * HID:]], 1)  # [64, 256]
        Wef2[0:64, l * 512:l * 512 + 256] = w
        Wef2[64:128, l * 512 + 256:(l + 1) * 512] = w
    biasfs = np.concatenate([np.concatenate([bf_[l], bs_[l]])[None]
                             for l in range(NCONV)], 1)        # [1, 4*256]
    poison = np.zeros((1, 256), BF16NP)
    poison[0, :HID] = POISON_VAL

    # layer-0 shortcut weights: P_src0 = x@(W_emb@W*_src) (+ b_emb@W*_src via
    # biasfs below); pad rows push f to -192 so sigmoid kills padded messages
    Wemb_f = np.asarray(inputs['W_emb'], np.float32)
    bemb_f = np.asarray(inputs['b_emb'], np.float32)
    Wsrc0 = Wsrc[:, 0:256]                      # [128, 256] layer-0 src proj
    WX = Wemb_f @ Wsrc0                         # [9, 256]
    Wxp = np.zeros((20, 512), np.float32)
    Wxp[0:9, 0:256] = WX
    Wxp[10:19, 256:512] = WX
    Wxp[9, 0:HID] = -192.0
    Wxp[19, 256:256 + HID] = -192.0
    biasfs[0, 0:256] += bemb_f @ Wsrc0          # fold b_emb@Wsrc into bias

    common = dict(
        W_emb=np.asarray(inputs['W_emb'], np.float32).astype(BF16NP),
        bemb_row=np.asarray(inputs['b_emb'], np.float32)[None, :],
        ones1=np.ones((1, 128), np.float32),
        Wsrc=Wsrc.astype(BF16NP), Wdst=Wdst.astype(BF16NP),
        Wef2=Wef2.astype(F8NP), biasfs=biasfs,
        Wxp=Wxp.astype(F8NP),
        gammaA=np.asarray(inputs['gamma'], np.float32).T.copy(),  # [128, 4]
        betaA=np.asarray(inputs['beta'], np.float32).T.copy(),
        W1=np.asarray(inputs['W1'], np.float32),
        b1=np.asarray(inputs['b1'], np.float32)[:, None],
        W2=np.pad(np.asarray(inputs['W2'], np.float32), ((0, 64), (0, 0))),
        b2=np.asarray(inputs['b2'], np.float32)[:, None],
        I128b=np.eye(128, dtype=BF16NP),
        I128f=np.eye(128, dtype=np.float32),
        poison=poison,
    )
    in_maps = []
    for c in range(NCORES):
        m = dict(common)
        m.update(xT=xT[c], eaT2=eaT2[c], gidx32=gidx32[c], invc=invc[c], xp2=xp2[c],
                 onehot=onehot[c], maskt=mask[c])
        in_maps.append(m)
    return meta, in_maps, percore, n_c


def _build(meta):
    import concourse.bass as bass
    import concourse.bacc as bacc
    import concourse.tile as tile
    from concourse import mybir
    from concourse import library_config

    F32 = mybir.dt.float32
    F16 = mybir.dt.float16
    BF = mybir.dt.bfloat16
    F8 = mybir.dt.float8e4
    I16 = mybir.dt.int16
    AF = mybir.ActivationFunctionType
    OP = mybir.AluOpType

    NPAD, NB, S = meta['NPAD'], meta['NB'], meta['S']
    Bi, Bi_lo, Bi_hi = meta['Bi'], meta['Bi_lo'], meta['Bi_hi']
    slot0, pslot0 = meta['slot0'], meta['pslot0']
    Bmax = max(Bi)
    R = NCORES * NPAD
    OFF = R - LO
    RG = [list(range(NCORES))]

    nc = bacc.Bacc("TRN2", target_bir_lowering=False, debug=False,
                   num_devices=NCORES)

    def P_(name, shape, dt):
        return nc.declare_dram_parameter(name, shape, dt, isOutput=False)

    xT_d = P_('xT', [9, NPAD], BF)
    eaT2_d = P_('eaT2', [128, (S // 2) * 128], F8)
    xp2_d = P_('xp2', [20, (S // 2) * 128], F8)
    gidx32_d = P_('gidx32', [128, S], mybir.dt.int32)
    invc_d = P_('invc', [128, NB], F32)
    onehot_d = P_('onehot', [128, NB * GPC], BF)
    mask_d = P_('maskt', [1, NPAD], BF)
    Wemb_d = P_('W_emb', [9, 128], BF)
    bembr_d = P_('bemb_row', [1, 128], F32)
    ones1_d = P_('ones1', [1, 128], F32)
    Wsrc_d = P_('Wsrc', [128, NCONV * 256], BF)
    Wdst_d = P_('Wdst', [128, NCONV * 256], BF)
    Wef2_d = P_('Wef2', [128, NCONV * 512], F8)
    Wxp_d = P_('Wxp', [20, 512], F8)
    biasfs_d = P_('biasfs', [1, NCONV * 256], F32)
    gammaA_d = P_('gammaA', [128, NCONV], F32)
    betaA_d = P_('betaA', [128, NCONV], F32)
    W1_d = P_('W1', [128, 64], F32)
    b1_d = P_('b1', [64, 1], F32)
    W2_d = P_('W2', [128, 1], F32)
    b2_d = P_('b2', [1, 1], F32)
    I128b_d = P_('I128b', [128, 128], BF)
    I128f_d = P_('I128f', [128, 128], F32)
    poison_d = P_('poison', [1, 256], BF)
    out_d = nc.declare_dram_parameter('outg', [1, GPC], F32, isOutput=True)

    with tile.TileContext(nc) as tc:
        with tc.tile_pool(name="res", bufs=1) as res, \
             tc.tile_pool(name="gp", bufs=2) as gp, \
             tc.tile_pool(name="wk", bufs=3) as wk, \
             tc.tile_pool(name="ea", bufs=2) as eap, \
             tc.tile_pool(name="ps", bufs=2, space="PSUM") as ps, \
             tc.tile_pool(name="psa", bufs=2, space="PSUM") as psa, \
             tc.tile_pool(name="pst", bufs=2, space="PSUM") as pstp, \
             tc.tile_pool(name="dram", bufs=1, space="DRAM") as dram:

            # (gpsimd library loads for InstDMAGatherAnt are auto-inserted
            # by Bacc.insert_library_loads at compile time)

            # ---- resident loads ----
            def load(shape, dt, d, tag):
                t = res.tile(shape, dt, tag=tag)
                nc.sync.dma_start(t[:], d[:])
                return t

            invc_sb = load([128, NB], F32, invc_d, 'invc')
            mask_sb = res.tile([128, NPAD], BF, tag='mask')
            nc.sync.dma_start(mask_sb[0:1, :], mask_d[:])
            nc.gpsimd.partition_broadcast(mask_sb[:], mask_sb[0:1, :])
            Wemb_sb = load([9, 128], BF, Wemb_d, 'wemb')
            bembr_sb = load([1, 128], F32, bembr_d, 'bembr')
            ones1_sb = load([1, 128], F32, ones1_d, 'ones1')
            Wsrcb_sb = load([128, NCONV * 256], BF, Wsrc_d, 'wsrcb')
            Wdstb_sb = load([128, NCONV * 256], BF, Wdst_d, 'wdstb')
            Wsrc_sb = res.tile([128, NCONV * 256], F32, tag='wsrc')
            nc.vector.tensor_copy(Wsrc_sb[:], Wsrcb_sb[:])
            Wdst_sb = res.tile([128, NCONV * 256], F32, tag='wdst')
            nc.vector.tensor_copy(Wdst_sb[:], Wdstb_sb[:])
            Wef2_sb = load([128, NCONV * 512], F8, Wef2_d, 'wef2')
            Wxp_sb = load([20, 512], F8, Wxp_d, 'wxp')
            biasfs_sb = load([1, NCONV * 256], F32, biasfs_d, 'biasfs')
            gammaA_sb = load([128, NCONV], F32, gammaA_d, 'gamA')
            betaA_sb = load([128, NCONV], F32, betaA_d, 'betA')
            W1_sb = load([128, 64], F32, W1_d, 'w1')
            b1_sb = load([64, 1], F32, b1_d, 'b1')
            W2_sb = load([128, 1], F32, W2_d, 'w2')
            b2_sb = load([1, 1], F32, b2_d, 'b2')
            I128b_sb = load([128, 128], BF, I128b_d, 'idb')
            I128f_sb = load([128, 128], F32, I128f_d, 'idf')
            poison_sb = load([1, 256], BF, poison_d, 'poi')

            h_loc = res.tile([128, NPAD], F32, tag='hloc')
            h_conv = res.tile([128, NPAD], F32, tag='hconv')
            Pdst_sb = res.tile([128, NB * 256], BF, tag='pdst')
            stats_sb = res.tile([128, 2], F32, tag='stats')
            scrg = res.tile([1, 8], mybir.dt.int32, tag='scrg')

            tbl_in = dram.tile([NPAD, 256], BF, tag='tblin')
            tbl_sh = dram.tile([R, 256], BF, tag='tblsh')
            stats_in = dram.tile([128, 2], F32, tag='stin')
            stats_out = dram.tile([128, 2], F32, tag='stout')

            zcol = res.tile([128, 1], F32, tag='zcol')
            nc.vector.memset(zcol[:], 0.0)
            nc.const_aps.aps[(F32, 0.0)] = zcol[:]
            ocol = res.tile([128, 1], F32, tag='ocol')
            nc.vector.memset(ocol[:], 1.0)
            nc.const_aps.aps[(F32, 1.0)] = ocol[:]

            # ---- embed ----
            for t in range(NB):
                xt = wk.tile([9, 128], BF, tag='xt')
                nc.sync.dma_start(xt[:], xT_d[:, t * 128:(t + 1) * 128])
                pe = ps.tile([128, 512], F32, tag='eps')
                nc.tensor.matmul(pe[:, 0:128], lhsT=Wemb_sb[:], rhs=xt[:],
                                 start=True, stop=False)
                nc.tensor.matmul(pe[:, 0:128], lhsT=bembr_sb[:], rhs=ones1_sb[:],
                                 start=False, stop=True)
                nc.vector.scalar_tensor_tensor(
                    out=h_loc[:, t * 128:(t + 1) * 128], in0=pe[:, 0:128],
                    scalar=1.0, in1=mask_sb[:, t * 128:(t + 1) * 128],
                    op0=OP.mult, op1=OP.mult)

            for l in range(NCONV):
                lc = slice(l * 256, (l + 1) * 256)
                # ---- node phase: src table slice + Pdst ----
                for t in range(NB):
                    hsl = h_loc[:, t * 128:(t + 1) * 128]
                    pn = ps.tile([128, 512], F32, tag='eps')
                    if l > 0:
                        nc.tensor.matmul(pn[:, 0:256], lhsT=hsl,
                                         rhs=Wsrc_sb[:, lc],
                                         start=True, stop=True)
                    nc.tensor.matmul(pn[:, 256:512], lhsT=hsl, rhs=Wdst_sb[:, lc],
                                     start=True, stop=False)
                    nc.tensor.matmul(pn[:, 256:512], lhsT=ones1_sb[:],
                                     rhs=biasfs_sb[:, lc], start=False, stop=True)
                    if l > 0:
                        st = wk.tile([128, 256], BF, tag='tstage')
                        nc.vector.tensor_copy(st[:], pn[:, 0:256])
                        nc.sync.dma_start(tbl_in[t * 128:(t + 1) * 128, :], st[:])
                    nc.vector.tensor_copy(Pdst_sb[:, t * 256:(t + 1) * 256],
                                          pn[:, 256:512])
                if l > 0:
                    nc.sync.dma_start(tbl_in[NPAD - 1:NPAD, :], poison_sb[:])
                    nc.gpsimd.collective_compute(
                        "AllGather", OP.bypass, replica_groups=RG,
                        ins=[tbl_in.opt()], outs=[tbl_sh.opt()])
                    tprobe = wk.tile([1, 128], BF, tag='tprobe')
                    nc.gpsimd.dma_start(tprobe[:], tbl_sh[0:1, 0:128])
                    nc.gpsimd.tensor_copy(scrg[0:1, 0:1].bitcast(BF),
                                          tprobe[0:1, 0:2])

                # ---- edge phase (blocks processed in pairs for ACT tables) -
                def head(i):
                    Bn = Bi[i]
                    npair = Bn // 2
                    s0 = slot0[i]
                    G = gp.tile([128, Bmax * 256], BF, tag='G')
                    G3 = G[:, 0:Bn * 256].rearrange("p (b d) -> p b d", b=Bn)
                    if l == 0:
                        # layer 0: src term comes from the xp2 matmul below;
                        # G starts as the Pdst broadcast
                        nc.vector.tensor_copy(
                            G3, Pdst_sb[:, i * 256:(i + 1) * 256].unsqueeze(1)
                            .to_broadcast([128, Bn, 256]))
                    else:
                        gx = wk.tile([128, Bmax], mybir.dt.int32, tag='gx32')
                        nc.sync.dma_start(gx[:, 0:Bn], gidx32_d[:, s0:s0 + Bn])
                        # per-slot indirect gather (overwrite); Pdst added after
                        for s in range(Bn):
                            nc.gpsimd.indirect_dma_start(
                                out=G[:, s * 256:(s + 1) * 256],
                                out_offset=None,
                                in_=tbl_sh[:, :],
                                in_offset=bass.IndirectOffsetOnAxis(
                                    ap=gx[:, s:s + 1], axis=0),
                                compute_op=OP.bypass)
                        # G += Pdst broadcast (bf16, DVE 4x)
                        nc.vector.scalar_tensor_tensor(
                            out=G3, in0=G3, scalar=1.0,
                            in1=Pdst_sb[:, i * 256:(i + 1) * 256].unsqueeze(1)
                            .to_broadcast([128, Bn, 256]),
                            op0=OP.mult, op1=OP.add)
                    ea_blk = eap.tile([128, (Bmax // 2) * 128], F8, tag='eab')
                    nc.sync.dma_start(
                        ea_blk[:, 0:npair * 128],
                        eaT2_d[:, pslot0[i] * 128:(pslot0[i] + npair) * 128])
                    if l == 0:
                        xp_blk = eap.tile([20, (Bmax // 2) * 128], F8, tag='xpb')
                        nc.sync.dma_start(
                            xp_blk[:, 0:npair * 128],
                            xp2_d[:, pslot0[i] * 128:(pslot0[i] + npair) * 128])
                    Gh = G[:].bitcast(F16)
                    for q in range(npair):
                        pe = ps.tile([128, 512], F32, tag='eps')
                        nc.tensor.matmul(pe[:],
                                         lhsT=ea_blk[:, q * 128:(q + 1) * 128],
                                         rhs=Wef2_sb[:, l * 512:(l + 1) * 512],
                                         start=True, stop=(l > 0))
                        if l == 0:
                            nc.tensor.matmul(
                                pe[:], lhsT=xp_blk[:, q * 128:(q + 1) * 128],
                                rhs=Wxp_sb[:], start=False, stop=True)
                        # G(f16) = pe + G(bf16): pre-activation f/s sums
                        nc.vector.scalar_tensor_tensor(
                            out=Gh[:, q * 512:(q + 1) * 512], in0=pe[:],
                            scalar=1.0, in1=G[:, q * 512:(q + 1) * 512],
                            op0=OP.mult, op1=OP.add)
                    sig_h = Gh.rearrange("p (s two d) -> p s two d",
                                         s=Bmax, two=2)[:, 0:Bn, 0, :]
                    e_h = Gh.rearrange("p (s two d) -> p s two d",
                                       s=Bmax, two=2)[:, 0:Bn, 1, :]
                    e_b = G[:].rearrange("p (s two d) -> p s two d",
                                         s=Bmax, two=2)[:, 0:Bn, 1, :]
                    # th = tanh(f/2) in f16 (the (1+th) form survives f16;
                    # bf16 would lose the near -1 cancellation)
                    nc.scalar.activation(sig_h, sig_h, AF.Tanh, scale=0.5)
                    # e = exp(s): f16 in, bf16 out (f16 would overflow at s>11)
                    nc.scalar.activation(e_b, e_h, AF.Exp)
                    return G, Gh

                def tail(i, G, Gh):
                    Bn = Bi[i]
                    e_b = G[:].rearrange("p (s two d) -> p s two d",
                                         s=Bmax, two=2)[:, 0:Bn, 1, :]
                    th_h = Gh.rearrange("p (s two d) -> p s two d",
                                        s=Bmax, two=2)[:, 0:Bn, 0, :]
                    msg_b = G[:].rearrange("p (s two d) -> p s two d",
                                           s=Bmax, two=2)[:, 0:Bn, 0, :]
                    # sp = ln(1 + e) (bf16)
                    nc.scalar.activation(e_b, e_b, AF.Ln, bias=1.0)
                    # msg2 = (th + 1) * sp -> bf16 at the f positions
                    nc.vector.scalar_tensor_tensor(
                        out=msg_b, in0=th_h, scalar=1.0, in1=e_b,
                        op0=OP.add, op1=OP.mult)
                    pag = psa.tile([128, 128], F32, tag='agg')
                    for s in range(Bn):
                        nc.tensor.matmul(pag[:], lhsT=I128b_sb[:],
                                         rhs=G[:, s * 256:s * 256 + 128],
                                         start=(s == 0), stop=(s == Bn - 1))
                    agn = wk.tile([128, 128], F32, tag='agn')
                    nc.vector.tensor_scalar_mul(agn[:], pag[:],
                                                invc_sb[:, i:i + 1])
                    ptr = pstp.tile([128, 128], F32, tag='tr')
                    nc.tensor.transpose(ptr[:], agn[:], I128f_sb[:])
                    nc.vector.tensor_tensor(
                        out=h_conv[:, i * 128:(i + 1) * 128], in0=ptr[:],
                        in1=h_loc[:, i * 128:(i + 1) * 128], op=OP.add)

                for ii in range(0, NB, 2):
                    g0 = head(ii)
                    g1 = head(ii + 1)
                    tail(ii, *g0)
                    tail(ii + 1, *g1)

                # ---- batch norm ----
                nc.vector.scalar_tensor_tensor(
                    out=h_conv[:], in0=h_conv[:], scalar=1.0, in1=mask_sb[:],
                    op0=OP.mult, op1=OP.mult)
                nc.vector.tensor_reduce(stats_sb[:, 0:1], h_conv[:],
                                        axis=mybir.AxisListType.X, op=OP.add)
                sqt = gp.tile([128, Bmax * 256], BF, tag='G')
                sqbuf = sqt[:].bitcast(F32)[:, 0:NPAD]
                nc.vector.scalar_tensor_tensor(
                    out=sqbuf, in0=h_conv[:], scalar=1.0, in1=h_conv[:],
                    op0=OP.mult, op1=OP.mult)
                nc.vector.tensor_reduce(stats_sb[:, 1:2], sqbuf,
                                        axis=mybir.AxisListType.X, op=OP.add)
                nc.sync.dma_start(stats_in[:], stats_sb[:])
                nc.gpsimd.collective_compute(
                    "AllReduce", OP.add, replica_groups=RG,
                    ins=[stats_in.opt()], outs=[stats_out.opt()])
                gst = wk.tile([128, 2], F32, tag='gst')
                nc.sync.dma_start(gst[:], stats_out[:])
                mu = wk.tile([128, 8], F32, tag='mu')
                nc.vector.tensor_scalar_mul(mu[:, 0:1], gst[:, 0:1], 1.0 / N)
                nc.vector.tensor_scalar_mul(mu[:, 1:2], gst[:, 1:2], 1.0 / N)
                nc.vector.tensor_tensor(out=mu[:, 2:3], in0=mu[:, 0:1],
                                        in1=mu[:, 0:1], op=OP.mult)
                nc.vector.tensor_tensor(out=mu[:, 3:4], in0=mu[:, 1:2],
                                        in1=mu[:, 2:3], op=OP.subtract)
                nc.vector.tensor_scalar_add(mu[:, 3:4], mu[:, 3:4], BN_EPS)
                nc.scalar.activation(mu[:, 4:5], mu[:, 3:4], AF.Ln)
                nc.scalar.activation(mu[:, 5:6], mu[:, 4:5], AF.Exp,
                                     scale=-0.5)
                nc.vector.tensor_tensor(out=mu[:, 6:7], in0=gammaA_sb[:, l:l + 1],
                                        in1=mu[:, 5:6], op=OP.mult)
                nc.vector.scalar_tensor_tensor(
                    out=mu[:, 7:8], in0=mu[:, 0:1], scalar=mu[:, 6:7],
                    op0=OP.mult, op1=OP.subtract, in1=betaA_sb[:, l:l + 1])
                nc.vector.tensor_scalar_mul(mu[:, 7:8], mu[:, 7:8], -1.0)
                nc.scalar.activation(h_conv[:], h_conv[:], AF.Relu,
                                     bias=mu[:, 7:8], scale=mu[:, 6:7])
                nc.vector.tensor_tensor(out=h_loc[:], in0=h_conv[:],
                                        in1=h_loc[:], op=OP.add)

            # ---- pooling + MLP ----
            ppool = psa.tile([128, GPC], F32, tag='pool')
            for i in range(NB):
                ptr = pstp.tile([128, 128], F32, tag='tr')
                nc.tensor.transpose(ptr[:], h_loc[:, i * 128:(i + 1) * 128],
                                    I128f_sb[:])
                hn = wk.tile([128, 128], BF, tag='hn')
                nc.vector.tensor_copy(hn[:], ptr[:])
                oht = wk.tile([128, GPC], BF, tag='oht')
                nc.sync.dma_start(oht[:], onehot_d[:, i * GPC:(i + 1) * GPC])
                nc.tensor.matmul(ppool[:], lhsT=hn[:], rhs=oht[:],
                                 start=(i == 0), stop=(i == NB - 1))
            pooled = wk.tile([128, GPC], F32, tag='pooled')
            nc.vector.tensor_copy(pooled[:], ppool[:])
            pz = ps.tile([128, 512], F32, tag='eps')
            nc.tensor.matmul(pz[0:64, 0:GPC], lhsT=W1_sb[:], rhs=pooled[:],
                             start=True, stop=True)
            z1 = wk.tile([64, GPC], F32, tag='z1')
            nc.scalar.activation(z1[:], pz[0:64, 0:GPC], AF.Exp,
                                 bias=b1_sb[:, 0:1])
            nc.scalar.activation(z1[:], z1[:], AF.Ln, bias=1.0)
            pz2 = pstp.tile([128, 128], F32, tag='tr')
            nc.tensor.matmul(pz2[0:1, 0:GPC], lhsT=W2_sb[0:64, :], rhs=z1[:],
                             start=True, stop=True)
            zo = wk.tile([1, GPC], F32, tag='zo')
            nc.scalar.activation(zo[:], pz2[0:1, 0:GPC], AF.Identity,
                                 bias=b2_sb[0:1, 0:1])
            nc.sync.dma_start(out_d[:], zo[:])

    nc.compile()
    return nc


TRACE = False
LAST_RESULTS = None


def kernel(**inputs):
    global LAST_RESULTS
    from concourse.bass_utils import run_bass_kernel_spmd

    meta, in_maps, percore, n_c = _host_prep(inputs)
    key = (meta['NPAD'], meta['S'], tuple(meta['Bi']), tuple(meta['Bi_lo']))
    if key not in _CACHE:
        _CACHE[key] = _build(meta)
    nc = _CACHE[key]
    res = run_bass_kernel_spmd(nc, in_maps, list(range(NCORES)), trace=TRACE)
    LAST_RESULTS = res
    out = np.concatenate([np.asarray(res.results[c]['outg']).reshape(GPC)
                          for c in range(NCORES)])
    return out.astype(np.float32)


def bench(inputs, reps=8):
    """Steady-state device timing: jit once, inputs device-resident, time
    repeated executes (async-pipelined, block at end)."""
    import time
    import jax
    from jax.sharding import Mesh, PartitionSpec
    from jax.experimental.shard_map import shard_map
    from concourse import bass2jax
    from concourse.bass2jax import _bass_exec_p, partition_id_tensor, \
        install_neuronx_cc_hook
    from concourse import mybir

    meta, in_maps, percore, n_c = _host_prep(inputs)
    key = (meta['NPAD'], meta['S'], tuple(meta['Bi']), tuple(meta['Bi_lo']))
    if key not in _CACHE:
        _CACHE[key] = _build(meta)
    nc = _CACHE[key]
    install_neuronx_cc_hook()
    n_cores = NCORES
    in_names, out_names, out_avals, zero_outs = [], [], [], []
    for alloc in nc.m.functions[0].allocations:
        if not isinstance(alloc, mybir.MemoryLocationSet):
            continue
        name = alloc.memorylocations[0].name
        pn = nc.partition_id_tensor.name if nc.partition_id_tensor else None
        if alloc.kind == "ExternalInput":
            if name != pn:
                in_names.append(name)
        elif alloc.kind == "ExternalOutput":
            out_names.append(name)
            shape = tuple(alloc.tensor_shape)
            dtype = mybir.dt.np(alloc.dtype)
            out_avals.append(jax.core.ShapedArray(shape, dtype))
            zero_outs.append(np.zeros(shape, dtype))
    n_params = len(in_names)
    n_outs = len(out_avals)
    all_names = list(in_names) + out_names
    pn = nc.partition_id_tensor.name if nc.partition_id_tensor else None
    if pn is not None:
        all_names.append(pn)

    def _body(*args):
        operands = list(args)
        if pn is not None:
            operands.append(partition_id_tensor())
        return tuple(_bass_exec_p.bind(
            *operands, out_avals=tuple(out_avals), in_names=tuple(all_names),
            out_names=tuple(out_names), lowering_input_output_aliases=(),
            sim_require_finite=True, sim_require_nnan=True, nc=nc))

    devices = jax.devices()[:n_cores]
    mesh = Mesh(np.asarray(devices), ("core",))
    in_specs = (PartitionSpec("core"),) * (n_params + n_outs)
    out_specs = (PartitionSpec("core"),) * len(out_names)
    sharded = jax.jit(shard_map(_body, mesh=mesh, in_specs=in_specs,
                                out_specs=out_specs, check_rep=False),
                      keep_unused=True)
    concat_in = [np.concatenate([np.asarray(in_maps[c][nm])
                                 for c in range(n_cores)], axis=0)
                 for nm in in_names]
    concat_zeros = [np.zeros((n_cores * z.shape[0], *z.shape[1:]), z.dtype)
                    for z in zero_outs]
    din = [jax.device_put(a) for a in concat_in]
    dzr = [jax.device_put(a) for a in concat_zeros]
    out = sharded(*din, *dzr)  # warmup + compile
    jax.block_until_ready(out)
    t0 = time.time()
    for _ in range(reps):
        out = sharded(*din, *dzr)
    jax.block_until_ready(out)
    dt = (time.time() - t0) / reps
    return dt, out


if __name__ == '__main__':
    import reference as ref
    inputs = {k: np.asarray(v) for k, v in ref.setup_inputs().items()}
    got = kernel(**inputs)
    exp = np.asarray(ref.reference(**ref.setup_inputs()))
    rel = np.abs(got - exp) / np.maximum(np.abs(exp), 1e-6)
    print('rel err max/mean:', rel.max(), rel.mean())

